# revision 1
# baseline (speedup 1.0000x reference)
"""PatchMatch-style MatchingPropagator on 8 Trainium2 NeuronCores.

Full inputs in, full outputs out. Sharding: 8 independent units =
(direction in {forward, backward}) x (batch 0..3), one NeuronCore each.

Key layout decisions:
- The host re-packs each unit's correlation volume into "quad" records
  Q[n, y0, x0, 0:4] = corr[n, y0:y0+2, x0:x0+2] for anchors in [0,62]^2,
  so every bilinear sample is ONE contiguous 16-byte indirect-DMA fetch.
  Clamping floors to <=62 is numerically identical to the reference's
  corner clamping.
- Every DVE op on the critical path reads/writes contiguous (or at most
  3-dim strided) access patterns; measured on TRN2, deep strided/broadcast
  views cost 2-3x a contiguous op of the same size.
- Candidate coords live in CC = [x-cols | y-cols] so floor/clamp/index
  ops are single wide contiguous ops; the [x|y|s] accept blocks in CT are
  filled by copies hidden under the gather's DMA flight time.
- The score uses prebuilt interleaved weight tiles UW = [u w u w] and
  TW = [t t wy wy] per pixel (built off the critical path), so the score
  is 2 contiguous multiplies + 3 stride-4 adds, bit-exact against the
  reference's product/sum order: s = ((t1+t2)+t3)+t4.
- The initial score eval is folded into the first propagate's gather
  (candidates pre-rolled on the host): 7 gathers total.

Pixel layout on chip: pixel (i, j) -> partition 64*(j//32) + i, free j%32.
"""

import numpy as np

B, H, W = 4, 64, 64
R = 3.0
EPS = np.float32(0.01)
N_CORES = 8
PIX = H * W              # 4096 pixels per unit
AN = W - 1               # 63 anchors per axis in the quad layout
QROW = AN * 4            # 252 floats per anchor row
QMAP = AN * AN * 4       # 15876 floats per pixel quad map
M_RNE = float(1 << 23)

_CACHE = {}


# ----------------------------------------------------------------------------
# Device program (SPMD: identical on all 8 cores; data differs per core)
# ----------------------------------------------------------------------------

def _build_program():
    import concourse.bass as bass
    import concourse.mybir as mybir
    import concourse.tile as tile
    from concourse import bacc

    F32 = mybir.dt.float32
    I32 = mybir.dt.int32
    OP = mybir.AluOpType
    AF = mybir.ActivationFunctionType

    nc = bacc.Bacc(
        "TRN2",
        target_bir_lowering=False,
        debug=False,
        enable_asserts=False,
        num_devices=N_CORES,
    )

    corr = nc.dram_tensor("corr", [PIX * QMAP], F32, kind="ExternalInput")
    # state cols (32 each): [x, y, hx1, hy1, vx1, vy1, base, nx1, ny1,
    #                        nx2, ny2, nx3, ny3]
    state_in = nc.dram_tensor("state", [128, 13 * 32], F32,
                              kind="ExternalInput")
    out_xy = nc.dram_tensor("out_xy", [128, 288], F32,
                            kind="ExternalOutput")

    corr_flat = corr.ap().rearrange("(n one) -> n one", one=1)

    def b3(ap):  # [128,32] -> broadcast [128,3,32]
        return ap.rearrange("p (one f) -> p one f", one=1).to_broadcast(
            [128, 3, 32])

    with tile.TileContext(nc) as tc:
        with tc.tile_pool(name="main", bufs=1) as pool:
            ST = pool.tile([128, 13 * 32], F32, name="ST")
            nc.gpsimd.dma_start(ST[:], state_in.ap())
            BASE = ST[:, 192:224]

            def noise_view(k):
                o = 224 + 64 * k
                return ST[:, o:o + 64]  # [nx|ny]

            # CT accept blocks of 96: [BEST | H | V], each [x|y|s]
            CT = pool.tile([128, 288], F32, name="CT")
            # candidate coords as [x y] pairs; separate tiles so the
            # v row-roll DMA never serializes against h-chain DVE writes
            CCH = pool.tile([128, 64], F32, name="CCH")
            CCV = pool.tile([128, 64], F32, name="CCV")
            G = pool.tile([128, 768], F32, name="G")
            UW = pool.tile([128, 384], F32, name="UW")   # [u w u w] per px
            TW = pool.tile([128, 384], F32, name="TW")   # [t t wy wy] per px
            WT = pool.tile([128, 192], F32, name="WT")   # [w | wy] per slot
            XI = pool.tile([128, 192], I32, name="XI")
            IF = pool.tile([128, 96], I32, name="IF")
            I = pool.tile([128, 96], I32, name="I")
            B1 = pool.tile([128, 384], F32, name="B1")
            B2 = pool.tile([128, 384], F32, name="B2")
            UPD = pool.tile([128, 128], I32, name="UPD")
            RCS = pool.tile([128, 192], F32, name="RCS")  # 3-variant RC
            WT3 = pool.tile([128, 192], F32, name="WT3")  # 3-variant [w|wy]
            RAV = pool.tile([128, 96], F32, name="RAV")  # rolled [xy|s] pre
            CTS = pool.tile([128, 96], F32, name="CTS")  # snapshot of BEST
            RBV = pool.tile([128, 96], F32, name="RBV")  # rolled [xy|s] RC
            XIS = pool.tile([128, 192], I32, name="XIS")
            ISF = pool.tile([128, 96], I32, name="ISF")
            IS = pool.tile([128, 96], I32, name="IS")
            BASEI = pool.tile([128, 32], I32, name="BASEI")
            v0 = nc.vector
            v0.memset(I[:, 0:32], 0)
            nc.gpsimd.indirect_dma_start(
                out=G[:, 0:128],
                out_offset=None,
                in_=corr_flat,
                in_offset=bass.IndirectOffsetOnAxis(ap=I[:, 0:32], axis=0),
            )
            v0.tensor_copy(BASEI[:], ST[:, 192:224])

            v = nc.vector

            def eval_pre(cv, ne, off):
                """floor + clamp + quad indices for an eval slot of `ne`
                candidates whose [x y]-pair coords are the contiguous view
                cv.  Floors via truncating f32->i32 cast (coords >= 0),
                clamped <= 61+1 in int.  Slot regions start at 32-col
                block `off`."""
                n = 64 * ne
                m = 32 * ne
                x0 = XI[:, 2 * 32 * off:2 * 32 * off + n]
                v.tensor_scalar(x0, cv, float(AN - 1), None, OP.min)
                x2 = x0.rearrange("p (c s q) -> p c s q", c=ne, s=2)
                if3 = IF[:, 32 * off:32 * off + m].rearrange(
                    "p (e q) -> p e q", e=ne)
                i3 = I[:, 32 * off:32 * off + m].rearrange(
                    "p (e q) -> p e q", e=ne)
                baseb = (BASEI.rearrange("p (one f) -> p one f", one=1)
                         .to_broadcast([128, ne, 32]))
                v.scalar_tensor_tensor(if3, x2[:, :, 1], QROW, baseb,
                                       OP.mult, OP.add)
                v.scalar_tensor_tensor(i3, x2[:, :, 0], 4, if3,
                                       OP.mult, OP.add)

            def eval_gather(ne, off):
                nc.gpsimd.indirect_dma_start(
                    out=G[:, 128 * off:128 * (off + ne)],
                    out_offset=None,
                    in_=corr_flat,
                    in_offset=bass.IndirectOffsetOnAxis(
                        ap=I[:, 32 * off:32 * (off + ne)], axis=0),
                )

            def eval_fill(ne, off):
                """interleave WT's [w|wy] into UW = [u w u w] and
                TW = [t t wy wy] per pixel."""
                m = 32 * ne
                wcol = WT[:, 64 * off:64 * off + m]
                wycol = WT[:, 64 * off + m:64 * off + 2 * m]
                uwv = UW[:, 128 * off:128 * (off + ne)].rearrange(
                    "p (e d s) -> p e d s", e=m, d=2, s=2)
                # (fraction tiles were computed by eval_weights/spec_rs)
                twv = TW[:, 128 * off:128 * (off + ne)].rearrange(
                    "p (e s d) -> p e s d", e=m, s=2, d=2)
                wb = (wcol.rearrange("p (e one) -> p e one", one=1)
                      .to_broadcast([128, m, 2]))
                wyb = (wycol.rearrange("p (e one) -> p e one", one=1)
                       .to_broadcast([128, m, 2]))
                v.tensor_copy(uwv[:, :, :, 1], wb)
                v.tensor_copy(twv[:, :, 1, :], wyb)
                v.tensor_scalar(uwv[:, :, :, 0], wb, -1.0, 1.0,
                                OP.mult, OP.add)
                v.tensor_scalar(twv[:, :, 0, :], wyb, -1.0, 1.0,
                                OP.mult, OP.add)

            def eval_weights(cv, ne, off):
                """hidden under gather flight: fraction compute + fill."""
                m = 32 * ne
                c2 = cv.rearrange("p (c s q) -> p c s q", c=ne, s=2)
                x2 = (XI[:, 64 * off:64 * off + 2 * m]
                      .rearrange("p (c s q) -> p c s q", c=ne, s=2))
                w = (WT[:, 64 * off:64 * off + m]
                     .rearrange("p (e q) -> p e q", e=ne))
                wy = (WT[:, 64 * off + m:64 * off + 2 * m]
                      .rearrange("p (e q) -> p e q", e=ne))
                v.tensor_tensor(w, c2[:, :, 0], x2[:, :, 0], OP.subtract)
                v.tensor_tensor(wy, c2[:, :, 1], x2[:, :, 1], OP.subtract)
                eval_fill(ne, off)

            def eval_score(ne, off, sc_dst):
                """bilinear score; bit-exact term/sum order of the
                reference: t_k = (corner*u_or_w)*t_or_wy,
                s = ((t1+t2)+t3)+t4 via a sequential innermost reduce."""
                lo, hi = 128 * off, 128 * (off + ne)
                v.tensor_tensor(B1[:, lo:hi], G[:, lo:hi], UW[:, lo:hi],
                                OP.mult)
                v.tensor_tensor(B2[:, lo:hi], B1[:, lo:hi], TW[:, lo:hi],
                                OP.mult)
                b4 = B2[:, lo:hi].rearrange("p (e k) -> p e k", k=4)
                v.tensor_reduce(sc_dst, b4, mybir.AxisListType.X, OP.add)

            def accept(blk, last=False):
                """BEST = candidate block blk where its score is higher.
                The decision mask is kept in UPD slot blk-1 so a following
                speculative random-search can select by it."""
                so = 96 * blk
                mo = 64 * (blk - 1)

                def b2(ap):
                    return (ap.rearrange("p (one f) -> p one f", one=1)
                            .to_broadcast([128, 2, 32]))

                v.tensor_tensor(UPD[:, mo:mo + 64].rearrange(
                    "p (c f) -> p c f", c=2), b2(CT[:, so + 64:so + 96]),
                    b2(CT[:, 64:96]), OP.is_gt)
                v.copy_predicated(CT[:, 0:64], UPD[:, mo:mo + 64],
                                  CT[:, so:so + 64])
                if not last:
                    v.copy_predicated(CT[:, 64:96], UPD[:, mo:mo + 32],
                                      CT[:, so + 64:so + 96])

            def spec_rs(k):
                """Speculative random-search index precompute, hidden under
                the current gather's flight: candidate coords + quad
                indices for each possible accept outcome (B, H, V).
                The base variant lands directly in I/CCH; the accepts'
                masks later select the H/V variants."""
                cv3 = (CT[:].rearrange("p (b f) -> p b f", b=3)[:, :, 0:64])
                nzb = (noise_view(k)
                       .rearrange("p (one f) -> p one f", one=1)
                       .to_broadcast([128, 3, 64]))
                rc3 = RCS[:].rearrange("p (c f) -> p c f", c=3)
                v.tensor_tensor(rc3, cv3, nzb, OP.add)
                v.tensor_scalar(RCS[:], RCS[:], 0.0, float(W - 1),
                                OP.max, OP.min)
                v.tensor_scalar(XIS[:], RCS[:], float(AN - 1), None, OP.min)
                x2 = XIS[:].rearrange("p (c s q) -> p c s q", c=3, s=2)
                if3 = ISF[:].rearrange("p (e q) -> p e q", e=3)
                i3 = IS[:].rearrange("p (e q) -> p e q", e=3)
                baseb = (BASEI.rearrange("p (one f) -> p one f", one=1)
                         .to_broadcast([128, 3, 32]))
                v.scalar_tensor_tensor(if3, x2[:, :, 1], QROW, baseb,
                                       OP.mult, OP.add)
                v.scalar_tensor_tensor(i3, x2[:, :, 0], 4, if3,
                                       OP.mult, OP.add)
                nc.gpsimd.indirect_dma_start(
                    out=G[:, 384:768],
                    out_offset=None,
                    in_=corr_flat,
                    in_offset=bass.IndirectOffsetOnAxis(ap=IS[:], axis=0),
                )
                w3 = WT3[:].rearrange("p (c s q) -> p c s q", c=3, s=2)
                rc2 = RCS[:].rearrange("p (c s q) -> p c s q", c=3, s=2)
                v.tensor_tensor(w3[:, :, 0], rc2[:, :, 0], x2[:, :, 0],
                                OP.subtract)
                v.tensor_tensor(w3[:, :, 1], rc2[:, :, 1], x2[:, :, 1],
                                OP.subtract)
                v.tensor_copy(WT[:, 0:64], WT3[:, 0:64])

            def sc_block(blk, nb=1):
                """CT score-column view [128, nb, 32] from block blk."""
                return (CT[:].rearrange("p (b f) -> p b f", b=3)
                        [:, blk:blk + nb, 64:96])

            # ---- round 1: initial eval + propagate(1,1); candidate coords
            # pre-rolled on the host.  Split into a (best,h) chain and a v
            # chain so the second gather's descriptor gen overlaps the
            # first's flight.
            eval_pre(ST[:, 0:192], 3, 0)
            eval_gather(3, 0)
            eval_weights(ST[:, 0:192], 3, 0)
            v.tensor_copy(CT[:, 0:64], ST[:, 0:64])
            v.tensor_copy(CT[:, 96:160], ST[:, 64:128])
            v.tensor_copy(CT[:, 192:256], ST[:, 128:192])
            spec_rs(0)
            eval_score(3, 0, sc_block(0, 3))
            accept(1)
            accept(2)

            def propagate(dx, dy, spec_k=None, last=False, rolled=True):
                # ---- h chain: col-roll by dx (DVE), clamp, idx
                dh = CCH[:].rearrange("p (c f) -> p c f", c=2)
                sh = CT[:, 0:64].rearrange("p (c f) -> p c f", c=2)
                if dx == 1:
                    v.tensor_copy(dh[:, :, 1:32], sh[:, :, 0:31])
                    v.tensor_copy(dh[64:128, :, 0:1], sh[0:64, :, 31:32])
                    v.tensor_copy(dh[0:64, :, 0:1], sh[64:128, :, 31:32])
                    v.tensor_scalar(CCH[:, 0:32], CCH[:, 0:32], 1.0,
                                    float(W - 1), OP.add, OP.min)
                else:
                    v.tensor_copy(dh[:, :, 0:31], sh[:, :, 1:32])
                    v.tensor_copy(dh[0:64, :, 31:32], sh[64:128, :, 0:1])
                    v.tensor_copy(dh[64:128, :, 31:32], sh[0:64, :, 0:1])
                    v.tensor_scalar(CCH[:, 0:32], CCH[:, 0:32], -1.0, 0.0,
                                    OP.add, OP.max)
                # ---- v chain: reconstruct the rolled BEST from the two
                # speculatively rolled variants (is_gt as early as its
                # rolled-score input can land), then the index chains
                def b2v(ap):
                    return (ap.rearrange("p (one f) -> p one f", one=1)
                            .to_broadcast([128, 2, 32]))

                v.tensor_tensor(UPD[:, 0:64].rearrange(
                    "p (c f) -> p c f", c=2), b2v(RBV[:, 64:96]),
                    b2v(RAV[:, 64:96]), OP.is_gt)
                v.copy_predicated(CCV[:], UPD[:, 0:64], RBV[:, 0:64])
                eval_pre(CCH[:], 1, 0)
                if dy == 1:
                    v.tensor_scalar(CCV[:, 32:64], CCV[:, 32:64], 1.0,
                                    float(H - 1), OP.add, OP.min)
                else:
                    v.tensor_scalar(CCV[:, 32:64], CCV[:, 32:64], -1.0,
                                    0.0, OP.add, OP.max)
                eval_pre(CCV[:], 1, 1)
                eval_gather(2, 0)

                # hidden work under the gather flight
                eval_weights(CCH[:], 1, 0)
                v.tensor_copy(CT[:, 96:160], CCH[:])
                eval_weights(CCV[:], 1, 1)
                v.tensor_copy(CT[:, 192:256], CCV[:])
                if spec_k is not None:
                    spec_rs(spec_k)
                eval_score(2, 0, sc_block(1, 2))
                if last:
                    return  # final accepts run on the host
                accept(1)
                accept(2)

            def roll_v(dst, src_ap, dy, q):
                """row-roll src [128,n] by dy into dst via 2 fused-AP DMAs
                on HWDGE queue q (0=sync, 1=scalar)."""
                eng = nc.sync if q == 0 else nc.scalar
                dvv = dst.rearrange("(b i) f -> b i f", b=2)
                svv = src_ap.rearrange("(b i) f -> b i f", b=2)
                if dy == 1:
                    eng.dma_start(dvv[:, 1:64], svv[:, 0:63])
                    eng.dma_start(dvv[:, 0:1], svv[:, 63:64])
                else:
                    eng.dma_start(dvv[:, 0:63], svv[:, 1:64])
                    eng.dma_start(dvv[:, 63:64], svv[:, 0:1])

            def random_search(k, next_dy):
                # corner data and weight fractions for all 3 variants were
                # prepared during the propagate's flight; select by the
                # accept masks, then score directly - no DMA on this path.
                # Both possible post-accept [x y s] blocks are row-rolled
                # speculatively so the next propagate's v-candidate needs
                # no DMA after the accept.  Roll from a snapshot so the
                # accept's writes to CT never wait on the roll's reads.
                v.tensor_copy(CTS[:], CT[:, 0:96])
                roll_v(RAV[:], CTS[:], next_dy, 1)

                def m4(mo):
                    return (UPD[:, mo:mo + 32]
                            .rearrange("p (q one) -> p q one", one=1)
                            .to_broadcast([128, 32, 4]))

                g4 = G[:, 384:512].rearrange("p (q k) -> p q k", k=4)
                gh = G[:, 512:640].rearrange("p (q k) -> p q k", k=4)
                gv = G[:, 640:768].rearrange("p (q k) -> p q k", k=4)
                v.copy_predicated(g4, m4(0), gh)
                v.copy_predicated(g4, m4(64), gv)
                v.copy_predicated(WT[:, 0:64], UPD[:, 0:64],
                                  WT3[:, 64:128])
                v.copy_predicated(WT[:, 0:64], UPD[:, 64:128],
                                  WT3[:, 128:192])
                v.tensor_copy(CT[:, 96:160], RCS[:, 0:64])
                v.copy_predicated(CT[:, 96:160], UPD[:, 0:64],
                                  RCS[:, 64:128])
                v.copy_predicated(CT[:, 96:160], UPD[:, 64:128],
                                  RCS[:, 128:192])
                roll_v(RBV[:, 0:64], CT[:, 96:160], next_dy, 0)
                eval_fill(1, 0)
                v.tensor_tensor(B1[:, 0:128], G[:, 384:512], UW[:, 0:128],
                                OP.mult)
                v.tensor_tensor(B2[:, 0:128], B1[:, 0:128], TW[:, 0:128],
                                OP.mult)
                b4 = B2[:, 0:128].rearrange("p (e k) -> p e k", k=4)
                v.tensor_reduce(sc_block(1), b4, mybir.AxisListType.X,
                                OP.add)
                roll_v(RBV[:, 64:96], CT[:, 160:192], next_dy, 1)
                v.tensor_copy(CCV[:], RAV[:, 0:64])
                accept(1)

            random_search(0, -1)
            propagate(-1, -1, spec_k=1)
            random_search(1, 1)
            propagate(-1, 1, spec_k=2)
            random_search(2, -1)
            propagate(1, -1, last=True)

            nc.sync.dma_start(out_xy.ap(), CT[:])

    nc.compile()
    return nc


def _get_program():
    if "nc" not in _CACHE:
        _CACHE["nc"] = _build_program()
    return _CACHE["nc"]


# ----------------------------------------------------------------------------
# Host-side helpers
# ----------------------------------------------------------------------------

def _to_layout(v):
    """[64(i), 64(j)] -> [128, 32]; partition = 64*(j//32)+i, free = j%32."""
    return np.ascontiguousarray(
        v.reshape(64, 2, 32).transpose(1, 0, 2).reshape(128, 32))


def _from_layout(a):
    """[128, 32] -> [64(i), 64(j)]."""
    return a.reshape(2, 64, 32).transpose(1, 0, 2).reshape(64, 64)


def _noise_arrays():
    """Mirror the reference's jax.random usage exactly, in-process, so the
    values match the grader's reference no matter which jax backend/PRNG
    the process defaults to."""
    import jax
    import jax.numpy as jnp

    key = jax.random.key(42)
    kf, kb = jax.random.split(key)
    out = []
    for kdir in (kf, kb):
        ks = jax.random.split(kdir, 3)
        out.append([np.asarray(R * jax.random.normal(k, (B, H, W, 2),
                                                     jnp.float32))
                    for k in ks])
    return out  # [dir][step] -> [B,H,W,2] float32


def _quad_pack(corr_u):
    """[4096, 64, 64] -> flat quad records [4096*63*63*4] f32."""
    sw = np.lib.stride_tricks.sliding_window_view(corr_u, (2, 2),
                                                  axis=(1, 2))
    # sw: [4096, 63, 63, 2, 2]
    return np.ascontiguousarray(sw).reshape(-1)


def _make_state(x_plane, y_plane, noise_steps, b):
    """Build the [128, 13*32] per-core state tensor (partition-major)."""
    x = x_plane.astype(np.float32)
    y = y_plane.astype(np.float32)
    one = np.float32(1.0)
    # first propagate is (dx, dy) = (1, 1); host pre-rolls the candidates
    hx = np.clip(np.roll(x, 1, axis=1) + one, np.float32(0.0),
                 np.float32(W - 1))
    hy = np.roll(y, 1, axis=1)
    vx = np.roll(x, 1, axis=0)
    vy = np.clip(np.roll(y, 1, axis=0) + one, np.float32(0.0),
                 np.float32(H - 1))
    base = ((np.arange(64, dtype=np.int64)[:, None] * 64
             + np.arange(64, dtype=np.int64)[None, :]) * QMAP)
    rows = [
        _to_layout(x), _to_layout(y),
        _to_layout(hx), _to_layout(hy),
        _to_layout(vx), _to_layout(vy),
        _to_layout(base.astype(np.float32)),
    ]
    for step in range(3):
        nz = noise_steps[step][b]  # [H,W,2]
        rows.append(_to_layout(np.ascontiguousarray(nz[:, :, 0])))
        rows.append(_to_layout(np.ascontiguousarray(nz[:, :, 1])))
    return np.concatenate(rows, axis=1).astype(np.float32)


def _bilinear_map_np(img, coords):
    """numpy mirror of reference._bilinear_map (fp32, same op order).
    img [B,H,W,C], coords [B,H,W,2] -> [B,H,W,C]"""
    Bn, Hn, Wn, C = img.shape
    out = np.empty_like(img)
    one = np.float32(1.0)
    for b in range(Bn):
        x = coords[b, :, :, 0].reshape(-1)
        y = coords[b, :, :, 1].reshape(-1)
        x0 = np.floor(x)
        y0 = np.floor(y)
        wx = (x - x0)[:, None]
        wy = (y - y0)[:, None]
        x0i = np.clip(x0.astype(np.int32), 0, Wn - 1)
        x1i = np.clip(x0i + 1, 0, Wn - 1)
        y0i = np.clip(y0.astype(np.int32), 0, Hn - 1)
        y1i = np.clip(y0i + 1, 0, Hn - 1)
        im = img[b]
        v00 = im[y0i, x0i]
        v01 = im[y0i, x1i]
        v10 = im[y1i, x0i]
        v11 = im[y1i, x1i]
        o = (v00 * (one - wx) * (one - wy) + v01 * wx * (one - wy)
             + v10 * (one - wx) * wy + v11 * wx * wy)
        out[b] = o.reshape(Hn, Wn, C)
    return out


def _run_device(in_maps, trace=False):
    from concourse import bass_utils

    nc = _get_program()
    res = bass_utils.run_bass_kernel_spmd(
        nc, in_maps, core_ids=list(range(N_CORES)), trace=trace)
    return res


def kernel(matching_f, matching_b, corr_map, _trace=False, _results_hook=None):
    matching_f = np.asarray(matching_f)
    matching_b = np.asarray(matching_b)
    corr_map = np.asarray(corr_map)

    noise = _noise_arrays()  # [dir][step][B,H,W,2]

    in_maps = []
    for b in range(B):  # forward units, cores 0..3
        corr_u = np.ascontiguousarray(corr_map[b]).reshape(PIX, H, W)
        in_maps.append({
            "corr": _quad_pack(corr_u),
            "state": _make_state(matching_f[b, 0], matching_f[b, 1],
                                 noise[0], b),
        })
    for b in range(B):  # backward units, cores 4..7
        corr_t = np.ascontiguousarray(
            corr_map[b].transpose(2, 3, 0, 1)).reshape(PIX, H, W)
        in_maps.append({
            "corr": _quad_pack(corr_t),
            "state": _make_state(matching_b[b, 0], matching_b[b, 1],
                                 noise[1], b),
        })

    res = _run_device(in_maps, trace=_trace)
    if _results_hook is not None:
        _results_hook(res)

    def _final_accepts(of):
        """host mirror of the last propagate's two sequential accepts."""
        xb, yb, sb = of[:, 0:32], of[:, 32:64], of[:, 64:96]
        xh, yh, sh = of[:, 96:128], of[:, 128:160], of[:, 160:192]
        xv, yv, sv = of[:, 192:224], of[:, 224:256], of[:, 256:288]
        u1 = sh > sb
        x1 = np.where(u1, xh, xb)
        y1 = np.where(u1, yh, yb)
        s1 = np.where(u1, sh, sb)
        u2 = sv > s1
        return np.where(u2, xv, x1), np.where(u2, yv, y1)

    res_f = np.empty((B, H, W, 2), np.float32)
    res_b = np.empty((B, H, W, 2), np.float32)
    for b in range(B):
        xf, yf = _final_accepts(res.results[b]["out_xy"])
        xb_, yb_ = _final_accepts(res.results[4 + b]["out_xy"])
        res_f[b, :, :, 0] = _from_layout(xf)
        res_f[b, :, :, 1] = _from_layout(yf)
        res_b[b, :, :, 0] = _from_layout(xb_)
        res_b[b, :, :, 1] = _from_layout(yb_)

    # forward-backward consistency (host; mirrors reference in fp32)
    counter = _bilinear_map_np(res_b, res_f)
    diff = np.max(np.abs(res_f - counter), axis=-1)
    invalid = (diff > EPS)[..., None]
    mf_t = matching_f.transpose(0, 2, 3, 1)  # [B,H,W,2]
    out = np.where(invalid, mf_t, res_f)
    return np.ascontiguousarray(out.transpose(0, 3, 1, 2)).astype(np.float32)



# revision 10
# speedup vs baseline: 1.2287x; 1.2287x over previous
"""PatchMatch-style MatchingPropagator on 8 Trainium2 NeuronCores.

Full inputs in, full outputs out. Sharding: 8 independent units =
(direction in {forward, backward}) x (batch 0..3), one NeuronCore each.

Key layout decisions:
- The host re-packs each unit's correlation volume into "quad" records
  Q[n, y0, x0, 0:4] = corr[n, y0:y0+2, x0:x0+2] for anchors in [0,62]^2,
  so every bilinear sample is ONE contiguous 16-byte indirect-DMA fetch.
  Clamping floors to <=62 is numerically identical to the reference's
  corner clamping.
- CT holds three [x|y|s] 96-col blocks (BEST, H-cand, V-cand); accepts
  are one 96-wide broadcast is_gt + one 96-wide copy_predicated.
- Bilinear weights are folded off the critical path into PW = UW*TW
  (the four per-corner weights), so a score eval on the critical path is
  one contiguous multiply + one [e,4] tensor_reduce with the reference's
  sequential sum order s = ((t1+t2)+t3)+t4.
- The random search is fully speculative: candidate coords, quad gather,
  weights AND scores (S3) are computed for all three possible propagate
  outcomes; after the accepts, two 96-wide predicated copies select the
  realized variant and one is_gt + copy_predicated applies the update.
- Neighbor candidates are produced by direct cross-partition DVE copies
  (partition-shifted access patterns), so no DMA sits on the
  accept -> roll -> next-eval critical chain.
- Weight interleaves ([u w u w], [t t wy wy]) run on the otherwise-idle
  Activation engine, hidden under the gather's DMA flight.

Pixel layout on chip: pixel (i, j) -> partition 64*(j//32) + i, free j%32.
"""

import numpy as np

B, H, W = 4, 64, 64
R = 3.0
EPS = np.float32(0.01)
N_CORES = 8
PIX = H * W              # 4096 pixels per unit
AN = W - 1               # 63 anchors per axis in the quad layout
QROW = AN * 4            # 252 floats per anchor row
QMAP = AN * AN * 4       # 15876 floats per pixel quad map

_CACHE = {}


# ----------------------------------------------------------------------------
# Device program (SPMD: identical on all 8 cores; data differs per core)
# ----------------------------------------------------------------------------

def _build_program():
    import concourse.bass as bass
    import concourse.mybir as mybir
    import concourse.tile as tile
    from concourse import bacc

    F32 = mybir.dt.float32
    I32 = mybir.dt.int32
    OP = mybir.AluOpType
    AF = mybir.ActivationFunctionType

    nc = bacc.Bacc(
        "TRN2",
        target_bir_lowering=False,
        debug=False,
        enable_asserts=False,
        num_devices=N_CORES,
    )

    corr = nc.dram_tensor("corr", [PIX * QMAP], F32, kind="ExternalInput")
    # state cols (32 each): [x, y, hx1, hy1, vx1, vy1, base, nx1, ny1,
    #                        nx2, ny2, nx3, ny3] + two 128-col permutation
    # matrices (row-roll +1 / -1) for the PE-based vertical roll
    state_in = nc.dram_tensor("state", [128, 13 * 32 + 256], F32,
                              kind="ExternalInput")
    out_xy = nc.dram_tensor("out_xy", [128, 288], F32,
                            kind="ExternalOutput")

    corr_flat = corr.ap().rearrange("(n one) -> n one", one=1)

    from concourse.bass import MemorySpace

    with tile.TileContext(nc) as tc:
        with tc.tile_pool(name="main", bufs=1) as pool, \
             tc.tile_pool(name="psum", bufs=2,
                          space=MemorySpace.PSUM) as ppool:
            ST = pool.tile([128, 13 * 32 + 256], F32, name="ST")
            nc.gpsimd.dma_start(ST[:], state_in.ap())
            PU = ST[:, 416:544]   # row-roll +1 permutation (lhsT)
            PD = ST[:, 544:672]   # row-roll -1 permutation (lhsT)

            def noise_view(k):
                o = 224 + 64 * k
                return ST[:, o:o + 64]  # [nx|ny]

            # CT: [BEST | H | V], each [x|y|s] 96 cols
            CT = pool.tile([128, 288], F32, name="CT")
            # candidate coords [xh yh | xv yv]
            CC = pool.tile([128, 128], F32, name="CC")
            G = pool.tile([128, 768], F32, name="G")     # eval 0:384, spec 384:768
            WT = pool.tile([128, 192], F32, name="WT")   # eval [w(<=96) | _ | wy@96]
            WT3 = pool.tile([128, 192], F32, name="WT3")  # spec [w96 | wy96]
            UW = pool.tile([128, 384], F32, name="UW")   # eval [u w u w] per px
            TW = pool.tile([128, 384], F32, name="TW")   # eval [t t wy wy] per px
            PW = pool.tile([128, 384], F32, name="PW")   # eval UW*TW
            UW3 = pool.tile([128, 384], F32, name="UW3")
            TW3 = pool.tile([128, 384], F32, name="TW3")
            PW3 = pool.tile([128, 384], F32, name="PW3")
            B2 = pool.tile([128, 384], F32, name="B2")   # eval products
            B3 = pool.tile([128, 384], F32, name="B3")   # spec products
            XI = pool.tile([128, 192], I32, name="XI")   # eval floored coords
            XIS = pool.tile([128, 192], I32, name="XIS")  # spec floored coords
            IF = pool.tile([128, 96], I32, name="IF")
            I = pool.tile([128, 96], I32, name="I")
            ISF = pool.tile([128, 96], I32, name="ISF")
            IS = pool.tile([128, 96], I32, name="IS")
            UPD = pool.tile([128, 192], I32, name="UPD")  # two 96-wide masks
            RC = pool.tile([128, 288], F32, name="RC")   # spec [x|y|s] x 3
            BASEI = pool.tile([128, 32], I32, name="BASEI")
            WI = pool.tile([128, 32], I32, name="WI")    # warm gather idx
            WG = pool.tile([128, 128], F32, name="WG")   # warm gather dest

            v = nc.vector
            a = nc.scalar

            # warm gather: triggers the gpsimd indirect-DMA library load
            # early; touches only dedicated tiles so nothing later stalls
            # on its completion.
            v.memset(WI[:], 0)
            nc.gpsimd.indirect_dma_start(
                out=WG[:],
                out_offset=None,
                in_=corr_flat,
                in_offset=bass.IndirectOffsetOnAxis(ap=WI[:], axis=0),
            )
            v.tensor_copy(BASEI[:], ST[:, 192:224])

            def b3(ap):  # [128,32] -> broadcast [128,3,32]
                return ap.rearrange("p (one f) -> p one f", one=1).to_broadcast(
                    [128, 3, 32])

            def ct_blk(i, n=1):
                """[128, n, 96] view of CT starting at block i."""
                return CT[:].rearrange("p (b f) -> p b f", b=3)[:, i:i + n]

            def rc_blk(i, n=1):
                return RC[:].rearrange("p (b f) -> p b f", b=3)[:, i:i + n]

            def eval_pre(cv, ne):
                """floor + clamp + quad indices for `ne` slots of candidate
                [x y]-pair coords in the contiguous view cv."""
                n = 64 * ne
                m = 32 * ne
                x0 = XI[:, 0:n]
                v.tensor_scalar(x0, cv, float(AN - 1), None, OP.min)
                x2 = x0.rearrange("p (c s q) -> p c s q", c=ne, s=2)
                if3 = IF[:, 0:m].rearrange("p (e q) -> p e q", e=ne)
                i3 = I[:, 0:m].rearrange("p (e q) -> p e q", e=ne)
                baseb = (BASEI.rearrange("p (one f) -> p one f", one=1)
                         .to_broadcast([128, ne, 32]))
                v.scalar_tensor_tensor(if3, x2[:, :, 1], QROW, baseb,
                                       OP.mult, OP.add)
                v.scalar_tensor_tensor(i3, x2[:, :, 0], 4, if3,
                                       OP.mult, OP.add)

            def eval_gather(ne):
                nc.gpsimd.indirect_dma_start(
                    out=G[:, 0:128 * ne],
                    out_offset=None,
                    in_=corr_flat,
                    in_offset=bass.IndirectOffsetOnAxis(
                        ap=I[:, 0:32 * ne], axis=0),
                )

            def weights_and_pw(cv, ne):
                """w/wy fractions (DVE), UW/TW interleave (Act), PW (DVE).
                All hidden under the eval gather's DMA flight."""
                m = 32 * ne
                c2 = cv.rearrange("p (c s q) -> p c s q", c=ne, s=2)
                x2 = (XI[:, 0:2 * m]
                      .rearrange("p (c s q) -> p c s q", c=ne, s=2))
                w = WT[:, 0:m].rearrange("p (e q) -> p e q", e=ne)
                wy = WT[:, 96:96 + m].rearrange("p (e q) -> p e q", e=ne)
                v.tensor_tensor(w, c2[:, :, 0], x2[:, :, 0], OP.subtract)
                v.tensor_tensor(wy, c2[:, :, 1], x2[:, :, 1], OP.subtract)
                fill(WT[:, 0:m], WT[:, 96:96 + m], UW, TW, m)
                v.tensor_tensor(PW[:, 0:4 * m], UW[:, 0:4 * m],
                                TW[:, 0:4 * m], OP.mult)

            def fill(wcol, wycol, uwt, twt, m):
                """interleave [w|wy] cols into uwt = [u w u w] and
                twt = [t t wy wy] per pixel, on the Activation engine."""
                uwv = uwt[:, 0:4 * m].rearrange(
                    "p (e d s) -> p e d s", e=m, d=2, s=2)
                twv = twt[:, 0:4 * m].rearrange(
                    "p (e s d) -> p e s d", e=m, s=2, d=2)
                wb = (wcol.rearrange("p (e one) -> p e one", one=1)
                      .to_broadcast([128, m, 2]))
                wyb = (wycol.rearrange("p (e one) -> p e one", one=1)
                       .to_broadcast([128, m, 2]))
                a.copy(uwv[:, :, :, 1], wb)
                a.copy(twv[:, :, 1, :], wyb)
                a.activation(uwv[:, :, :, 0], wb, AF.Identity, bias=1.0,
                             scale=-1.0)
                a.activation(twv[:, :, 0, :], wyb, AF.Identity, bias=1.0,
                             scale=-1.0)

            def eval_score(ne):
                """score for `ne` eval slots -> CT s-cols of blocks H.. ."""
                n = 128 * ne
                v.tensor_tensor(B2[:, 0:n], G[:, 0:n], PW[:, 0:n], OP.mult)
                b4 = B2[:, 0:n].rearrange("p (e k) -> p e k", k=4)
                sc = ct_blk(3 - ne, ne)[:, :, 64:96]
                v.tensor_reduce(sc, b4, mybir.AxisListType.X, OP.add)

            def accept(blk):
                """BEST = block blk where its score is higher; one 96-wide
                mask + one 96-wide predicated copy. Mask lands in UPD slot
                blk-1 for the random-search variant selection."""
                mo = 96 * (blk - 1)
                m3 = UPD[:, mo:mo + 96].rearrange("p (c f) -> p c f", c=3)
                v.tensor_tensor(m3, b3(ct_blk(blk)[:, 0, 64:96]),
                                b3(CT[:, 64:96]), OP.is_gt)
                v.copy_predicated(CT[:, 0:96], UPD[:, mo:mo + 96],
                                  ct_blk(blk)[:, 0])

            def spec_idx_gather(k, first=False):
                """Speculative random-search for all three possible accept
                outcomes: coords + indices + gather, issued right behind
                the eval gather. Weights follow in spec_weights; scores
                (S3) in rs_finish once the gather lands."""
                nzb = (noise_view(k)
                       .rearrange("p (one f) -> p one f", one=1)
                       .to_broadcast([128, 3, 64]))
                rxy = rc_blk(0, 3)[:, :, 0:64]
                if first:
                    # candidates live in ST [x,y,hx,hy,vx,vy]
                    cv3 = ST[:, 0:192].rearrange("p (c f) -> p c f", c=3)
                    v.tensor_tensor(rxy, cv3, nzb, OP.add)
                else:
                    # B from CT, H/V from CC
                    v.tensor_tensor(rc_blk(0)[:, :, 0:64],
                                    ct_blk(0)[:, :, 0:64], nzb[:, 0:1],
                                    OP.add)
                    cc2 = CC[:].rearrange("p (c f) -> p c f", c=2)
                    v.tensor_tensor(rc_blk(1, 2)[:, :, 0:64], cc2,
                                    nzb[:, 0:2], OP.add)
                v.tensor_scalar(rxy, rxy, 0.0, float(W - 1),
                                OP.max, OP.min)
                v.tensor_scalar(XIS[:], rxy, float(AN - 1), None, OP.min)
                x2 = XIS[:].rearrange("p (c s q) -> p c s q", c=3, s=2)
                if3 = ISF[:].rearrange("p (e q) -> p e q", e=3)
                i3 = IS[:].rearrange("p (e q) -> p e q", e=3)
                baseb = (BASEI.rearrange("p (one f) -> p one f", one=1)
                         .to_broadcast([128, 3, 32]))
                v.scalar_tensor_tensor(if3, x2[:, :, 1], QROW, baseb,
                                       OP.mult, OP.add)
                v.scalar_tensor_tensor(i3, x2[:, :, 0], 4, if3,
                                       OP.mult, OP.add)
                nc.gpsimd.indirect_dma_start(
                    out=G[:, 384:768],
                    out_offset=None,
                    in_=corr_flat,
                    in_offset=bass.IndirectOffsetOnAxis(ap=IS[:], axis=0),
                )

            def spec_weights():
                """spec weight fractions + interleave + PW3, hidden under
                the gathers' flight."""
                rxy = rc_blk(0, 3)[:, :, 0:64]
                x2 = XIS[:].rearrange("p (c s q) -> p c s q", c=3, s=2)
                w3 = WT3[:, 0:96].rearrange("p (c q) -> p c q", c=3)
                wy3 = WT3[:, 96:192].rearrange("p (c q) -> p c q", c=3)
                rc2 = rxy.rearrange("p c (s q) -> p c s q", s=2)
                v.tensor_tensor(w3, rc2[:, :, 0], x2[:, :, 0], OP.subtract)
                v.tensor_tensor(wy3, rc2[:, :, 1], x2[:, :, 1], OP.subtract)
                fill(WT3[:, 0:96], WT3[:, 96:192], UW3, TW3, 96)
                v.tensor_tensor(PW3[:], UW3[:], TW3[:], OP.mult)

            def rs_finish():
                """Score all three spec variants, select the realized one by
                the accept masks, apply the random-search update."""
                v.tensor_tensor(B3[:], G[:, 384:768], PW3[:], OP.mult)
                b4 = B3[:].rearrange("p (e k) -> p e k", k=4)
                s3 = rc_blk(0, 3)[:, :, 64:96]
                v.tensor_reduce(s3, b4, mybir.AxisListType.X, OP.add)
                # select realized variant into RC block 0
                v.copy_predicated(RC[:, 0:96], UPD[:, 0:96], rc_blk(1)[:, 0])
                v.copy_predicated(RC[:, 0:96], UPD[:, 96:192],
                                  rc_blk(2)[:, 0])
                # accept: new_s > old_s
                m3 = UPD[:, 0:96].rearrange("p (c f) -> p c f", c=3)
                v.tensor_tensor(m3, b3(RC[:, 64:96]), b3(CT[:, 64:96]),
                                OP.is_gt)
                v.copy_predicated(CT[:, 0:96], UPD[:, 0:96], RC[:, 0:96])

            def rolls(dx, dy):
                """H/V propagate candidates from CT best into
                CC = [xh yh | xv yv]. The vertical (row) roll is a
                partition shift: an exact permutation matmul on the idle
                PE engine; the horizontal roll shifts the free dim on DVE
                in parallel."""
                src = CT[:, 0:64]
                # ---- v: permutation matmul (partition roll) -> PSUM
                ps = ppool.tile([128, 64], F32)
                nc.tensor.matmul(ps[:], PU if dy == 1 else PD, src,
                                 start=True, stop=True)
                # ---- h: roll cols (free shift + cross-half wrap) on DVE
                dh = CC[:, 0:64].rearrange("p (c f) -> p c f", c=2)
                sh = src.rearrange("p (c f) -> p c f", c=2)
                if dx == 1:
                    v.tensor_copy(dh[:, :, 1:32], sh[:, :, 0:31])
                    v.tensor_copy(dh[64:128, :, 0:1], sh[0:64, :, 31:32])
                    v.tensor_copy(dh[0:64, :, 0:1], sh[64:128, :, 31:32])
                    v.tensor_scalar(CC[:, 0:32], CC[:, 0:32], 1.0,
                                    float(W - 1), OP.add, OP.min)
                else:
                    v.tensor_copy(dh[:, :, 0:31], sh[:, :, 1:32])
                    v.tensor_copy(dh[0:64, :, 31:32], sh[64:128, :, 0:1])
                    v.tensor_copy(dh[64:128, :, 31:32], sh[0:64, :, 0:1])
                    v.tensor_scalar(CC[:, 0:32], CC[:, 0:32], -1.0, 0.0,
                                    OP.add, OP.max)
                # ---- v tail: copy x, clamp y out of PSUM
                v.tensor_copy(CC[:, 64:96], ps[:, 0:32])
                if dy == 1:
                    v.tensor_scalar(CC[:, 96:128], ps[:, 32:64], 1.0,
                                    float(H - 1), OP.add, OP.min)
                else:
                    v.tensor_scalar(CC[:, 96:128], ps[:, 32:64], -1.0,
                                    0.0, OP.add, OP.max)

            def ct_save():
                """CC -> CT H/V coord cols (Act, under gather flight)."""
                a.copy(CT[:, 96:160], CC[:, 0:64])
                a.copy(CT[:, 192:256], CC[:, 64:128])

            # ---- round 1: initial eval + propagate(1,1); candidates
            # pre-rolled on the host in ST.
            eval_pre(ST[:, 0:192], 3)
            eval_gather(3)
            # CT init: [x|y] of B/H/V from ST's 3 coord pairs
            cxy = (CT[:].rearrange("p (b f) -> p b f", b=3)[:, :, 0:64])
            v.tensor_copy(cxy, ST[:, 0:192].rearrange(
                "p (c f) -> p c f", c=3))
            spec_idx_gather(0, first=True)
            weights_and_pw(ST[:, 0:192], 3)
            spec_weights()
            eval_score(3)
            accept(1)
            accept(2)
            rs_finish()

            def propagate(dx, dy, spec_k=None):
                rolls(dx, dy)
                eval_pre(CC[:], 2)
                eval_gather(2)
                # hidden under the gather flight
                if spec_k is not None:
                    spec_idx_gather(spec_k)
                weights_and_pw(CC[:], 2)
                ct_save()
                if spec_k is not None:
                    spec_weights()
                eval_score(2)
                if spec_k is None:
                    return  # final accepts run on the host
                accept(1)
                accept(2)
                rs_finish()

            propagate(-1, -1, spec_k=1)
            propagate(-1, 1, spec_k=2)
            propagate(1, -1)

            nc.sync.dma_start(out_xy.ap(), CT[:])

    nc.compile()
    return nc


def _get_program():
    if "nc" not in _CACHE:
        _CACHE["nc"] = _build_program()
    return _CACHE["nc"]


# ----------------------------------------------------------------------------
# Host-side helpers
# ----------------------------------------------------------------------------

def _to_layout(v):
    """[64(i), 64(j)] -> [128, 32]; partition = 64*(j//32)+i, free = j%32."""
    return np.ascontiguousarray(
        v.reshape(64, 2, 32).transpose(1, 0, 2).reshape(128, 32))


def _from_layout(a):
    """[128, 32] -> [64(i), 64(j)]."""
    return a.reshape(2, 64, 32).transpose(1, 0, 2).reshape(64, 64)


def _noise_arrays():
    """Mirror the reference's jax.random usage exactly, in-process, so the
    values match the grader's reference no matter which jax backend/PRNG
    the process defaults to."""
    import jax
    import jax.numpy as jnp

    key = jax.random.key(42)
    kf, kb = jax.random.split(key)
    out = []
    for kdir in (kf, kb):
        ks = jax.random.split(kdir, 3)
        out.append([np.asarray(R * jax.random.normal(k, (B, H, W, 2),
                                                     jnp.float32))
                    for k in ks])
    return out  # [dir][step] -> [B,H,W,2] float32


def _quad_pack(corr_u):
    """[4096, 64, 64] -> flat quad records [4096*63*63*4] f32."""
    sw = np.lib.stride_tricks.sliding_window_view(corr_u, (2, 2),
                                                  axis=(1, 2))
    # sw: [4096, 63, 63, 2, 2]
    return np.ascontiguousarray(sw).reshape(-1)


def _roll_perm_mats():
    """Permutation lhsT matrices for the PE row-roll: out[m] = src[sig(m)]
    with sig(m) = 64*(m//64) + ((m%64 -/+ 1) % 64)."""
    up = np.zeros((128, 128), np.float32)
    dn = np.zeros((128, 128), np.float32)
    for m in range(128):
        blk, i = divmod(m, 64)
        up[64 * blk + (i - 1) % 64, m] = 1.0
        dn[64 * blk + (i + 1) % 64, m] = 1.0
    return up, dn


def _make_state(x_plane, y_plane, noise_steps, b):
    """Build the [128, 13*32+256] per-core state tensor (partition-major)."""
    x = x_plane.astype(np.float32)
    y = y_plane.astype(np.float32)
    one = np.float32(1.0)
    # first propagate is (dx, dy) = (1, 1); host pre-rolls the candidates
    hx = np.clip(np.roll(x, 1, axis=1) + one, np.float32(0.0),
                 np.float32(W - 1))
    hy = np.roll(y, 1, axis=1)
    vx = np.roll(x, 1, axis=0)
    vy = np.clip(np.roll(y, 1, axis=0) + one, np.float32(0.0),
                 np.float32(H - 1))
    base = ((np.arange(64, dtype=np.int64)[:, None] * 64
             + np.arange(64, dtype=np.int64)[None, :]) * QMAP)
    rows = [
        _to_layout(x), _to_layout(y),
        _to_layout(hx), _to_layout(hy),
        _to_layout(vx), _to_layout(vy),
        _to_layout(base.astype(np.float32)),
    ]
    for step in range(3):
        nz = noise_steps[step][b]  # [H,W,2]
        rows.append(_to_layout(np.ascontiguousarray(nz[:, :, 0])))
        rows.append(_to_layout(np.ascontiguousarray(nz[:, :, 1])))
    rows.extend(_roll_perm_mats())
    return np.concatenate(rows, axis=1).astype(np.float32)


def _bilinear_map_np(img, coords):
    """numpy mirror of reference._bilinear_map (fp32, same op order).
    img [B,H,W,C], coords [B,H,W,2] -> [B,H,W,C]"""
    Bn, Hn, Wn, C = img.shape
    out = np.empty_like(img)
    one = np.float32(1.0)
    for b in range(Bn):
        x = coords[b, :, :, 0].reshape(-1)
        y = coords[b, :, :, 1].reshape(-1)
        x0 = np.floor(x)
        y0 = np.floor(y)
        wx = (x - x0)[:, None]
        wy = (y - y0)[:, None]
        x0i = np.clip(x0.astype(np.int32), 0, Wn - 1)
        x1i = np.clip(x0i + 1, 0, Wn - 1)
        y0i = np.clip(y0.astype(np.int32), 0, Hn - 1)
        y1i = np.clip(y0i + 1, 0, Hn - 1)
        im = img[b]
        v00 = im[y0i, x0i]
        v01 = im[y0i, x1i]
        v10 = im[y1i, x0i]
        v11 = im[y1i, x1i]
        o = (v00 * (one - wx) * (one - wy) + v01 * wx * (one - wy)
             + v10 * (one - wx) * wy + v11 * wx * wy)
        out[b] = o.reshape(Hn, Wn, C)
    return out


def _run_device(in_maps, trace=False):
    from concourse import bass_utils

    nc = _get_program()
    res = bass_utils.run_bass_kernel_spmd(
        nc, in_maps, core_ids=list(range(N_CORES)), trace=trace)
    return res


def kernel(matching_f, matching_b, corr_map, _trace=False, _results_hook=None):
    matching_f = np.asarray(matching_f)
    matching_b = np.asarray(matching_b)
    corr_map = np.asarray(corr_map)

    noise = _noise_arrays()  # [dir][step][B,H,W,2]

    in_maps = []
    for b in range(B):  # forward units, cores 0..3
        corr_u = np.ascontiguousarray(corr_map[b]).reshape(PIX, H, W)
        in_maps.append({
            "corr": _quad_pack(corr_u),
            "state": _make_state(matching_f[b, 0], matching_f[b, 1],
                                 noise[0], b),
        })
    for b in range(B):  # backward units, cores 4..7
        corr_t = np.ascontiguousarray(
            corr_map[b].transpose(2, 3, 0, 1)).reshape(PIX, H, W)
        in_maps.append({
            "corr": _quad_pack(corr_t),
            "state": _make_state(matching_b[b, 0], matching_b[b, 1],
                                 noise[1], b),
        })

    res = _run_device(in_maps, trace=_trace)
    if _results_hook is not None:
        _results_hook(res)

    def _final_accepts(of):
        """host mirror of the last propagate's two sequential accepts."""
        xb, yb, sb = of[:, 0:32], of[:, 32:64], of[:, 64:96]
        xh, yh, sh = of[:, 96:128], of[:, 128:160], of[:, 160:192]
        xv, yv, sv = of[:, 192:224], of[:, 224:256], of[:, 256:288]
        u1 = sh > sb
        x1 = np.where(u1, xh, xb)
        y1 = np.where(u1, yh, yb)
        s1 = np.where(u1, sh, sb)
        u2 = sv > s1
        return np.where(u2, xv, x1), np.where(u2, yv, y1)

    res_f = np.empty((B, H, W, 2), np.float32)
    res_b = np.empty((B, H, W, 2), np.float32)
    for b in range(B):
        xf, yf = _final_accepts(res.results[b]["out_xy"])
        xb_, yb_ = _final_accepts(res.results[4 + b]["out_xy"])
        res_f[b, :, :, 0] = _from_layout(xf)
        res_f[b, :, :, 1] = _from_layout(yf)
        res_b[b, :, :, 0] = _from_layout(xb_)
        res_b[b, :, :, 1] = _from_layout(yb_)

    # forward-backward consistency (host; mirrors reference in fp32)
    counter = _bilinear_map_np(res_b, res_f)
    diff = np.max(np.abs(res_f - counter), axis=-1)
    invalid = (diff > EPS)[..., None]
    mf_t = matching_f.transpose(0, 2, 3, 1)  # [B,H,W,2]
    out = np.where(invalid, mf_t, res_f)
    return np.ascontiguousarray(out.transpose(0, 3, 1, 2)).astype(np.float32)
